# revision 43
# baseline (speedup 1.0000x reference)
"""Trainium2 Bass kernel for nn_AttentionBlock (b=4, c=512, h=w=64).

Sharding: 8 cores = (batch 0..3) x (sequence half 0..1). Each core receives
its batch's x [512, 4096] ROTATED so that the core's query half occupies
local columns 0:2048 (attention is permutation-invariant over keys, and
groupnorm stats are order-invariant, so one SPMD program serves all cores).

Per-core pipeline (fp8e4 + DoubleRow on the PE, [128,1024] "super" PSUM
tiles spanning two banks so every PSUM consumer op covers two matmul
outputs at once — halves the per-instruction overhead on ACT/DVE):
  A) x loaded ONCE into resident SBUF tiles [128, 2048] x 8, as 16
     256KB half-tile DMAs spread over the sync/gpsimd/scalar queues (all
     16 HW DMA engines serve every queue; 2KB rows keep them efficient;
     the ACT queue gets few pushes since descriptor-ring stalls would
     head-of-line block its stats ops).  Groupnorm stats: DVE bn_stats
     on 22 chunks + ACT Identity/Square accum on 5 early-arriving
     halves; ONE gmask matmul does the group reduce+broadcast.  A PE
     warmup stream (dense burst + arrival-paced trickle + finalize
     blips) wins the HAM full-clock grant by ~12us and holds it, so the
     QKV stream starts at speed; dummy Sqrt/Exp activations keep the
     ACT table loads off the critical path.
  B) normalize x from SBUF into fp8 pair tiles [128, 2, 1024] on GPSIMD
     (1-input ops are line-rate there; DVE/ACT stay free for PSUM work),
     then QKV as fp8 DoubleRow matmuls into supers: K/Q converted by DVE
     scalar_tensor_tensor (x1/16 + per-pair broadcast bias), V by ACT
     Identity (x1/16), each one [128,1024] op per super.
  C) per 512-query chunk: scores super = two S^T tiles (4 DR matmuls),
     ONE exp ACT [128,1024] (scale=1/sqrt(c), bias=-1.5; shift cancels in
     softmax, keeps E under fp8e4 max 240) -> fp8 E pair tiles; U
     accumulates in two supers over 16 key-tile pairs; Z on DVE (fp8 E
     adds into f32 [128,2,512]), folded + column-summed/broadcast via a
     (1/8)-valued f32r matmul (folds the x8 attn prescale), attn = U*(8/Z)
     via rbb broadcast to both super halves; proj DR matmuls into the
     scores ring; post-proj DVE stt applies 1/128 + pair-broadcast bias,
     residual added from the resident x tiles.  The per-qc tail (Z fold,
     attn, proj, store) is deferred into the NEXT qc's score stream so the
     PE never idles on the serial Z chain; the Z chains start with an ADD
     of two E tiles (no slow fp8->f32 CAST), pps drains via ACT+DVE+GPSIMD
     in parallel, and the final chunk hoists its Z/reciprocal ahead of the
     last PV matmuls and fans its stores out over 3 queues.
HBM traffic: x read once (8MB), weights ~1MB, out 4MB. No DRAM scratch.
"""

import os
import sys
from contextlib import ExitStack

for _p in ("/opt/trn_rl_repo", "/root/.axon_site/_ro/trn_rl_repo"):
    if os.path.isdir(_p) and _p not in sys.path:
        sys.path.insert(0, _p)

import numpy as np
import ml_dtypes

import concourse.bass as bass
import concourse.tile as tile
from concourse import bacc, mybir
from concourse.bass_utils import run_bass_kernel_spmd

F32 = mybir.dt.float32
F32R = mybir.dt.float32r
BF16 = mybir.dt.bfloat16
F8 = mybir.dt.float8e4
NP8 = ml_dtypes.float8_e4m3
NPBF = ml_dtypes.bfloat16
ALU = mybir.AluOpType
ACT = mybir.ActivationFunctionType
DR = mybir.MatmulPerfMode.DoubleRow

N_CORES = 8
C = 512          # channels
N = 4096         # h*w
NQ = 2048        # queries per core
CT = C // 128    # 4 channel tiles
NCHUNK = N // 512   # 8 column chunks
QCHUNK = NQ // 512  # 4 query chunks per core
MT = N // 128    # 32 key tiles
PRS = MT // 2    # 16 key-tile pairs
GSIZE = 16       # channels per group
EPS = 1e-5
WSCALE = 16.0    # host-side fp8 weight prescale
ASCALE = 8.0     # attn prescale (folded into the Z broadcast matmul)
ESHIFT = -1.5    # exp shift; cancels in softmax
SCALE_QK = 1.0 / float(np.sqrt(np.float32(C)))
NORM_ON_GPSIMD = True
WARM_MMS = 32    # phase-A PE warmup matmuls (HAM full-clock grant)


def build_module(reps: int = 1, use_pbias: bool = True):
    nc = bacc.Bacc("TRN2", target_bir_lowering=False, debug=False,
                   num_devices=N_CORES)

    xin = nc.dram_tensor("xin", [C, N], BF16, kind="ExternalInput").ap()
    w8 = nc.dram_tensor("w8", [C, 3 * C], F8, kind="ExternalInput").ap()
    wpd = nc.dram_tensor("wpd", [C, C], F8, kind="ExternalInput").ap()
    cvec = nc.dram_tensor("cvec", [128, 28], F32, kind="ExternalInput").ap()
    gmask = nc.dram_tensor("gmask", [128, 128], F32,
                           kind="ExternalInput").ap()
    out = nc.dram_tensor("out", [C, NQ], F32, kind="ExternalOutput").ap()

    with tile.TileContext(nc) as tc, \
            nc.allow_low_precision(reason="fp8 attention by design"):
        for rep in range(reps):
            _emit_body(tc, rep, xin, w8, wpd, cvec, gmask, out,
                       use_pbias)
    nc.compile()
    return nc


def _emit_body(tc, rep, xin, w8, wpd, cvec, gmask, out, use_pbias=True):
    nc = tc.nc
    norm_eng = nc.gpsimd if NORM_ON_GPSIMD else nc.vector
    with ExitStack() as ctx:
        # ---- persistent pools ----
        const = ctx.enter_context(tc.tile_pool(name=f"const{rep}", bufs=1))
        # one pool per resident-x tile: consumers then wait only on that
        # tile's two DMAs instead of the whole 16-DMA batch
        xpools = [ctx.enter_context(tc.tile_pool(name=f"xres{rep}_{i}",
                                                 bufs=1)) for i in range(8)]
        kpool = ctx.enter_context(tc.tile_pool(name=f"kbuf{rep}", bufs=1))
        vpool = ctx.enter_context(tc.tile_pool(name=f"vbuf{rep}", bufs=1))
        qpool = ctx.enter_context(tc.tile_pool(name=f"qbuf{rep}", bufs=1))
        wpool = ctx.enter_context(tc.tile_pool(name=f"wgt{rep}", bufs=1))
        statp = ctx.enter_context(tc.tile_pool(name=f"stat{rep}", bufs=1))

        # resident x: xa[ct*2 + jj] = [128, 2048] bf16 (cols jj*2048...)
        xa = [xpools[i].tile([128, 2048], BF16, name=f"xa{i}", tag=f"xa{i}")
              for i in range(8)]
        # fp8 K: pair p (channels 256p..256p+255), column chunk j
        K_f8 = [[kpool.tile([128, 2, 512], F8, name=f"K{p}_{j}",
                            tag=f"K{p}_{j}") for j in range(NCHUNK)]
                for p in range(2)]
        # fp8 V^T: key-tile pair pr, free = 512 channels
        V_f8 = [vpool.tile([128, 2, 512], F8, name=f"V{pr}", tag=f"V{pr}")
                for pr in range(PRS)]
        # fp8 Q: query chunk qc, channel pair p
        Q_f8 = [[qpool.tile([128, 2, 512], F8, name=f"Q{qc}_{p}",
                            tag=f"Q{qc}_{p}") for p in range(2)]
                for qc in range(QCHUNK)]
        # fp8 weights: qkv pair tiles and proj pair tiles
        w3 = [wpool.tile([128, 2, 3 * C], F8, name=f"w3_{p}", tag=f"w3_{p}")
              for p in range(2)]
        wp8 = [wpool.tile([128, 2, C], F8, name=f"wp{p}", tag=f"wp{p}")
               for p in range(2)]

        cvec_t = const.tile([128, 28], F32, name="cvec", tag="cvec")
        gmask_t = const.tile([128, 128], F32, name="gmask", tag="gmask")
        ones_mat_f = const.tile([128, 128], F32, name="onemf", tag="onemf")
        nc.vector.memset(ones_mat_f[:], 1.0 / ASCALE)
        ones_mat = const.tile([128, 128], F32R, name="onem", tag="onem")
        nc.vector.tensor_copy(ones_mat[:], ones_mat_f[:])
        eps_t = const.tile([128, 1], F32, name="epst", tag="epst")
        nc.vector.memset(eps_t[:], EPS)
        esh_t = const.tile([128, 1], F32, name="esht", tag="esht")
        nc.vector.memset(esh_t[:], ESHIFT)
        # warmup-matmul operand + dummy-op scratch
        warm_t = const.tile([128, 512], BF16, name="warmt", tag="warmt")
        nc.vector.memset(warm_t[:], 0.0)
        dum_t = const.tile([128, 8], F32, name="dumt", tag="dumt")
        # pull the set-0 ACT table load (Identity/Square/Exp) to the very
        # start, before the first stats op needs it
        nc.scalar.activation(out=dum_t[:, 2:3], in_=eps_t[:],
                             func=ACT.Identity, scale=1.0)
        # cvec layout: [bq x4][bk x4][bp x4][gnw x4][gnb x4][indr x8]
        bq_t = [cvec_t[:, ct:ct + 1] for ct in range(CT)]
        bp_t = [cvec_t[:, 8 + ct:9 + ct] for ct in range(CT)]
        gnw_t = [cvec_t[:, 12 + ct:13 + ct] for ct in range(CT)]
        gnb_t = [cvec_t[:, 16 + ct:17 + ct] for ct in range(CT)]
        indr_t = cvec_t[:, 20:28]
        # pair bias tiles [128, 2, 1] for free-dim broadcast over supers
        bq2 = [const.tile([128, 2, 1], F32, name=f"bq2_{p}", tag=f"bq2_{p}")
               for p in range(2)]
        bk2 = [const.tile([128, 2, 1], F32, name=f"bk2_{p}", tag=f"bk2_{p}")
               for p in range(2)]
        bp2 = [const.tile([128, 2, 1], F32, name=f"bp2_{p}", tag=f"bp2_{p}")
               for p in range(2)]

        scale_ca = statp.tile([128, CT], F32, name="sca", tag="sca")
        bias_ca = statp.tile([128, CT], F32, name="bca", tag="bca")
        scale_c = [scale_ca[:, ct:ct + 1] for ct in range(CT)]
        bias_c = [bias_ca[:, ct:ct + 1] for ct in range(CT)]

        # ================= Phase A: load x + groupnorm statistics ========
        with ExitStack() as pa:
            scpa = pa.enter_context(tc.tile_pool(name=f"sca{rep}", bufs=2))
            tmpa = pa.enter_context(tc.tile_pool(name=f"tmpa{rep}", bufs=2))
            psa = pa.enter_context(
                tc.tile_pool(name=f"psa{rep}", bufs=2, space="PSUM"))

            # ---- DMA issue.  Every queue's packets are serviced by all 16
            # HW DMA engines, so queue count doesn't add bandwidth; what
            # matters is pushing descriptors early, keeping x bytes ahead
            # of weight bytes, and keeping the DVE/ACT queues free of
            # pushes (descriptor rings fill and head-of-line block the
            # engine).  x tiles are split 512/512/1024 so stats start on
            # the first 128KB piece.
            def xpiece(i, lo, hi, q):
                q.dma_start(xa[i][:, lo:hi],
                            xin[(i // 2) * 128:(i // 2 + 1) * 128,
                                (i % 2) * 2048 + lo:(i % 2) * 2048 + hi])

            nc.sync.dma_start(cvec_t[:], cvec)
            nc.sync.dma_start(gmask_t[:], gmask)
            # 256KB pieces (2KB rows: smaller pieces tank DMA efficiency).
            # Bytes balanced ~1MB/1.5MB/1.5MB across scalar/gpsimd/sync —
            # the 16 HW engines round-robin across queues, so the heaviest
            # queue sets the last-arrival time.  The scalar (ACT) queue
            # carries the ACT-owned pieces; its 4th push stalls on the
            # ~4-deep descriptor ring but only until piece 1 lands.
            for i in (0, 2, 4, 6):
                xpiece(i, 0, 1024, nc.scalar)
            for i, h in ((3, 0), (1, 0), (5, 0), (7, 0), (1, 1), (3, 1)):
                xpiece(i, h * 1024, (h + 1) * 1024, nc.gpsimd)
            for i, h in ((0, 1), (2, 1), (4, 1), (6, 1), (5, 1), (7, 1)):
                xpiece(i, h * 1024, (h + 1) * 1024, nc.sync)
            # weights strictly after x (first QKV needs w3 at ~28us)
            for p in range(2):
                for s in range(2):
                    r0 = p * 256 + s * 128
                    nc.gpsimd.dma_start(w3[p][:, s, :], w8[r0:r0 + 128, :])
                    nc.sync.dma_start(wp8[p][:, s, :], wpd[r0:r0 + 128, :])
            # GPSIMD ucode (IRAM) warm: touch both op libraries while idle
            nc.gpsimd.tensor_scalar(out=dum_t[:, 4:5], in0=eps_t[:],
                                    scalar1=1.0, scalar2=0.0,
                                    op0=ALU.mult, op1=ALU.add)
            nc.gpsimd.tensor_add(dum_t[:, 5:6], eps_t[:], eps_t[:])

            # ---- PE warmup: a dense burst triggers the HAM full-clock
            # grant (~6.5us of sustained activity), then a trickle paced
            # by the arriving x pieces holds it (re-throttle window is
            # only ~2.5us) until the real QKV stream begins.
            def warm_mm(moving=None):
                wps = psa.tile([128, 512], F32, name="wps", tag="wps")
                nc.tensor.matmul(wps[:], warm_t[:, 0:128],
                                 warm_t[:] if moving is None else moving,
                                 start=True, stop=True)

            for _ in range(WARM_MMS):
                warm_mm()

            # ---- stats ownership: ACT takes the [0:1024] half of each
            # jj0 tile plus ct1's jj1 [0:1024] (5 big halves, arriving
            # earliest — ACT is the slower engine); DVE bn_stats the
            # remaining 22 chunks.  n_dve cols per ct: 3072/2048/3072/
            # 3072 of 4096.
            DVE_OFF = (0, 6, 10, 16)    # stats slot offset per ct
            stats = statp.tile([128, 22, 6], F32, name="bst", tag="bst")
            sacc1 = statp.tile([128, CT, 2], F32, name="sa1", tag="sa1")
            sacc2 = statp.tile([128, CT, 2], F32, name="sa2", tag="sa2")
            nc.vector.memset(sacc1[:], 0.0)
            nc.vector.memset(sacc2[:], 0.0)

            def stat_piece(i, h):
                ct, jj = i // 2, i % 2
                xt = xa[i]
                lo = h * 1024
                if h == 0 and (jj == 0 or i == 3):     # ACT-owned half
                    kc = 0 if jj == 0 else 1
                    scr = scpa.tile([128, 1024], BF16, name="scr",
                                    tag="scr")
                    nc.scalar.activation(
                        out=scr[:], in_=xt[:, 0:1024],
                        func=ACT.Identity, accum_out=sacc1[:, ct, kc:kc + 1])
                    nc.scalar.activation(
                        out=scr[:], in_=xt[:, 0:1024],
                        func=ACT.Square, accum_out=sacc2[:, ct, kc:kc + 1])
                else:
                    base = DVE_OFF[ct]
                    if jj == 0:
                        slot = base            # h==1 only
                    elif h == 0:
                        slot = base + 2
                    else:
                        slot = base + 2 if ct == 1 else base + 4
                    for c2 in range(2):
                        nc.vector.bn_stats(
                            out=stats[:, slot + c2, :],
                            in_=xt[:, lo + c2 * 512:lo + (c2 + 1) * 512])
                # warmup trickle paced by this piece's arrival
                warm_mm(xt[:, lo:lo + 512])
                warm_mm(xt[:, lo + 256:lo + 768])
                warm_mm(xt[:, lo + 512:lo + 1024])

            # emission in approximate arrival order (per-queue FIFO)
            arrival = [(3, 0), (0, 0), (0, 1), (1, 0), (2, 0), (2, 1),
                       (5, 0), (4, 0), (4, 1), (7, 0), (6, 0), (6, 1),
                       (1, 1), (5, 1), (3, 1), (7, 1)]
            for i, h in arrival:
                stat_piece(i, h)
            # bridge warmups: keep the PE fed between the last trickle and
            # the finalize matmuls (~25us -> ~31us); they gate nothing —
            # the group-reduce matmul waits on the DVE finalize chain
            # anyway
            for _ in range(3):
                for i in (1, 5, 3, 7):
                    warm_mm(xa[i][:, 1536:2048])
            # dummy Sqrt: pulls the set-1 ACT table load off the critical
            # path (it runs right after the last stats op, overlapping the
            # DVE finalize chain, so the real Sqrt finds it resident)
            nc.scalar.activation(out=dum_t[:, 0:1], in_=eps_t[:],
                                 func=ACT.Sqrt, scale=1.0)

            # ---- finalize: combine bn_aggr partials with ACT sums per
            # ct: mean = (512*n_dve*mean_A + S1)/4096, same for E[x^2].
            ndve_t = tmpa.tile([128, CT], F32, name="ndve", tag="ndve")
            for ct, w in enumerate((0.75, 0.5, 0.75, 0.75)):
                nc.vector.memset(ndve_t[:, ct:ct + 1], w)
            # t12a layout: cols [mean x4 | E[x^2] x4]
            t12a = tmpa.tile([128, 2 * CT], F32, name="t12a", tag="t12a")
            mva = tmpa.tile([128, CT, 2], F32, name="mva", tag="mva")
            for ct in range(CT):
                n_sl = (DVE_OFF + (22,))[ct + 1] - DVE_OFF[ct]
                nc.vector.bn_aggr(
                    out=mva[:, ct, :],
                    in_=stats[:, DVE_OFF[ct]:DVE_OFF[ct] + n_sl, :])
            s1t = tmpa.tile([128, CT], F32, name="s1t", tag="s1t")
            s2t = tmpa.tile([128, CT], F32, name="s2t", tag="s2t")
            nc.vector.tensor_reduce(out=s1t[:], in_=sacc1[:],
                                    axis=mybir.AxisListType.X,
                                    op=ALU.add)
            nc.vector.tensor_reduce(out=s2t[:], in_=sacc2[:],
                                    axis=mybir.AxisListType.X,
                                    op=ALU.add)
            # mean = w_ct*mean_A + S1/4096
            nc.vector.tensor_mul(t12a[:, 0:CT], mva[:, :, 0], ndve_t[:])
            nc.vector.scalar_tensor_tensor(
                out=t12a[:, 0:CT], in0=s1t[:], scalar=1.0 / N,
                in1=t12a[:, 0:CT], op0=ALU.mult, op1=ALU.add)
            # EX2 = w_ct*(var_A + mean_A^2) + S2/4096
            ex2 = tmpa.tile([128, CT], F32, name="ex2", tag="ex2")
            nc.vector.tensor_mul(ex2[:], mva[:, :, 0], mva[:, :, 0])
            nc.vector.tensor_add(ex2[:], ex2[:], mva[:, :, 1])
            nc.vector.tensor_mul(t12a[:, CT:2 * CT], ex2[:], ndve_t[:])
            nc.vector.scalar_tensor_tensor(
                out=t12a[:, CT:2 * CT], in0=s2t[:], scalar=1.0 / N,
                in1=t12a[:, CT:2 * CT], op0=ALU.mult, op1=ALU.add)
            # group reduce + broadcast in ONE matmul: gmask[p,q] =
            # 1/16 if group(p)==group(q), so cs = gmask^T @ t12a gives the
            # per-group means already broadcast to every channel
            cps = psa.tile([128, 2 * CT], F32, name="cps", tag="cps")
            nc.tensor.matmul(cps[:], gmask_t[:], t12a[:], start=True,
                             stop=True)
            for _ in range(3):
                warm_mm(xa[7][:, 1536:2048])
            cs = tmpa.tile([128, 2 * CT], F32, name="cs", tag="cs")
            nc.vector.tensor_copy(cs[:], cps[:])
            # var = E[x^2] - mean^2 ; rstd = 1/sqrt(var+eps)  (all 4 tiles)
            var_t = tmpa.tile([128, CT], F32, name="var", tag="var")
            nc.vector.tensor_mul(var_t[:], cs[:, 0:CT], cs[:, 0:CT])
            nc.vector.tensor_sub(var_t[:], cs[:, CT:2 * CT], var_t[:])
            blip = psa.tile([8, CT], F32, name="blip", tag="blip")
            nc.tensor.matmul(blip[:], gmask_t[:, 0:8], var_t[:],
                             start=True, stop=True)
            for _ in range(2):
                warm_mm(xa[5][:, 1536:2048])
            sq_t = tmpa.tile([128, CT], F32, name="sq", tag="sq")
            nc.scalar.activation(out=sq_t[:], in_=var_t[:],
                                 func=ACT.Sqrt, bias=eps_t[:], scale=1.0)
            # dummy Exp: reloads set-0 (Identity/Exp) during the DVE
            # reciprocal/scale chain, before the normalize Identity ops
            nc.scalar.activation(out=dum_t[:, 1:2], in_=eps_t[:],
                                 func=ACT.Exp, scale=1.0)
            rstd_t = tmpa.tile([128, CT], F32, name="rstd", tag="rstd")
            nc.vector.reciprocal(rstd_t[:], sq_t[:])
            nc.vector.tensor_mul(scale_ca[:], rstd_t[:], cvec_t[:, 12:16])
            blip2 = psa.tile([8, CT], F32, name="blip", tag="blip")
            nc.tensor.matmul(blip2[:], gmask_t[:, 0:8], scale_ca[:],
                             start=True, stop=True)
            for _ in range(2):
                warm_mm(xa[3][:, 1536:2048])
            mt_t = tmpa.tile([128, CT], F32, name="mt", tag="mt")
            nc.vector.tensor_mul(mt_t[:], cs[:, 0:CT], scale_ca[:])
            nc.vector.tensor_sub(bias_ca[:], cvec_t[:, 16:20], mt_t[:])
            blip3 = psa.tile([8, CT], F32, name="blip", tag="blip")
            nc.tensor.matmul(blip3[:], gmask_t[:, 0:8], bias_ca[:],
                             start=True, stop=True)

        # ======= Phases B + C share the scores machinery: chunk qc=0's
        # score pairs are emitted inside phase B right after each K chunk
        # is produced, merging B and C into one continuous PE stream =====
        pbc = ctx.enter_context(ExitStack())
        epool = pbc.enter_context(tc.tile_pool(name=f"e{rep}", bufs=20))
        ps_s = pbc.enter_context(
            tc.tile_pool(name=f"pss{rep}", bufs=2, space="PSUM"))
        e_store = {}

        def scores_pair_g(qc, pr):
            ss = ps_s.tile([128, 1024], F32, name="s", tag="s")
            for i2 in range(2):
                mt = 2 * pr + i2
                for p in range(2):
                    nc.tensor.matmul(
                        ss[:, i2 * 512:(i2 + 1) * 512],
                        K_f8[p][mt // 4][
                            :, :, (mt % 4) * 128:(mt % 4 + 1) * 128],
                        Q_f8[qc][p][:], start=(p == 0), stop=(p == 1),
                        perf_mode=DR)
            e = epool.tile([128, 2, 512], F8, name="e", tag="e")
            nc.scalar.activation(
                out=e[:], in_=ss[:], func=ACT.Exp,
                bias=esh_t[:], scale=SCALE_QK)
            e_store[(qc, pr)] = e

        # ================= Phase B: normalize + QKV (fp8 DR supers) ======
        with ExitStack() as pb:
            xbp = pb.enter_context(tc.tile_pool(name=f"xb{rep}", bufs=2))
            psb = pb.enter_context(
                tc.tile_pool(name=f"psb{rep}", bufs=2, space="PSUM"))

            for jp in range(NCHUNK // 2):
                xf8 = []
                for p in range(2):
                    xt = xbp.tile([128, 2, 1024], F8, name=f"xf{p}",
                                  tag=f"xf{p}")
                    for s in range(2):
                        ct = 2 * p + s
                        src = xa[ct * 2 + jp // 2][
                            :, (jp % 2) * 1024:(jp % 2) * 1024 + 1024]
                        # first chunk is latency-critical: spread its 4
                        # converts over DVE+GPSIMD (p0, gating the first K
                        # matmul) and ACT (p1); GPSIMD's ucode was warmed
                        # by the phase-A dummies
                        if jp == 0 and p == 0:
                            eng = nc.vector if s == 0 else nc.gpsimd
                            eng.tensor_scalar(
                                out=xt[:, s, :], in0=src,
                                scalar1=scale_c[ct],
                                scalar2=bias_c[ct],
                                op0=ALU.mult, op1=ALU.add)
                        elif jp == 0:
                            nc.scalar.activation(
                                out=xt[:, s, :], in_=src,
                                func=ACT.Identity,
                                bias=bias_c[ct], scale=scale_c[ct])
                        else:
                            norm_eng.tensor_scalar(
                                out=xt[:, s, :], in0=src,
                                scalar1=scale_c[ct],
                                scalar2=bias_c[ct],
                                op0=ALU.mult, op1=ALU.add)
                    xf8.append(xt)
                if jp == 0:
                    # pair-bias broadcast tiles: DVE is idle here and the
                    # first consumer (K-super stt) is ~2us away
                    for p in range(2):
                        nc.vector.tensor_copy(bq2[p][:],
                                              cvec_t[:, 2 * p:2 * p + 2])
                        nc.vector.tensor_copy(
                            bk2[p][:], cvec_t[:, 4 + 2 * p:6 + 2 * p])
                        nc.vector.tensor_copy(
                            bp2[p][:], cvec_t[:, 8 + 2 * p:10 + 2 * p])

                for jh in range(2):
                    j = jp * 2 + jh
                    xn = [xf8[p][:, :, jh * 512:(jh + 1) * 512]
                          for p in range(2)]
                    # K supers: halves (ot=2h, 2h+1) -> K_f8[h][j]
                    for h in range(2):
                        ks = psb.tile([128, 1024], F32, name="sup",
                                      tag="sup")
                        for s in range(2):
                            ot = 2 * h + s
                            for p in range(2):
                                nc.tensor.matmul(
                                    ks[:, s * 512:(s + 1) * 512],
                                    w3[p][:, :,
                                          C + ot * 128:C + (ot + 1) * 128],
                                    xn[p], start=(p == 0), stop=(p == 1),
                                    perf_mode=DR)
                        nc.vector.scalar_tensor_tensor(
                            out=K_f8[h][j][:], in0=ks[:],
                            scalar=1.0 / WSCALE,
                            in1=bk2[h][:].to_broadcast((128, 2, 512)),
                            op0=ALU.mult, op1=ALU.add)
                    # V supers: halves mt=(4j+2i, 4j+2i+1) -> V_f8[2j+i]
                    for i in range(2):
                        pr = 2 * j + i
                        vs = psb.tile([128, 1024], F32, name="sup",
                                      tag="sup")
                        for s in range(2):
                            mti = 2 * i + s
                            for p in range(2):
                                nc.tensor.matmul(
                                    vs[:, s * 512:(s + 1) * 512],
                                    xn[p][:, :, mti * 128:(mti + 1) * 128],
                                    w3[p][:, :, 2 * C:3 * C],
                                    start=(p == 0), stop=(p == 1),
                                    perf_mode=DR)
                        if j >= NCHUNK - 2:
                            # keep ACT's FIFO clear near the end of phase
                            # B so the first exp isn't queued behind it
                            nc.vector.tensor_scalar_mul(
                                V_f8[pr][:], vs[:], 1.0 / WSCALE)
                        else:
                            nc.scalar.activation(
                                out=V_f8[pr][:], in_=vs[:],
                                func=ACT.Identity, scale=1.0 / WSCALE)
                    # Q supers (only local columns 0:2048 are queries)
                    if j < QCHUNK:
                        for h in range(2):
                            qs = psb.tile([128, 1024], F32, name="sup",
                                          tag="sup")
                            for s in range(2):
                                ot = 2 * h + s
                                for p in range(2):
                                    nc.tensor.matmul(
                                        qs[:, s * 512:(s + 1) * 512],
                                        w3[p][:, :,
                                              ot * 128:(ot + 1) * 128],
                                        xn[p], start=(p == 0), stop=(p == 1),
                                        perf_mode=DR)
                            nc.vector.scalar_tensor_tensor(
                                out=Q_f8[j][h][:], in0=qs[:],
                                scalar=1.0 / WSCALE,
                                in1=bq2[h][:].to_broadcast((128, 2, 512)),
                                op0=ALU.mult, op1=ALU.add)
                    # qc=0 scores for the PREVIOUS chunk's key tiles: the
                    # K_f8 conversion (DVE stt) lands ~1.5us after the K
                    # supers, so scoring the fresh chunk here would stall
                    # LDWEIGHTS on the conversion
                    if j > 0:
                        scores_pair_g(0, 2 * (j - 1))
                        scores_pair_g(0, 2 * j - 1)
            # last chunk's qc0 scores (shifted out of the loop above)
            scores_pair_g(0, 2 * (NCHUNK - 1))
            scores_pair_g(0, 2 * NCHUNK - 1)

        # ================= Phase C: attention + proj (fp8 DR supers) =====
        with ExitStack() as pc:
            apool = pc.enter_context(tc.tile_pool(name=f"at{rep}", bufs=2))
            outp = pc.enter_context(tc.tile_pool(name=f"out{rep}", bufs=3))
            miscp = pc.enter_context(tc.tile_pool(name=f"mi{rep}", bufs=2))
            ps_u = pc.enter_context(
                tc.tile_pool(name=f"psu{rep}", bufs=1, space="PSUM"))

            pending_zb = None
            pending_proj = None
            for qc in range(QCHUNK):
                def scores_pair(pr, qc=qc):
                    scores_pair_g(qc, pr)

                # 8 score pairs head start (qc=0's were emitted in phase
                # B); the previous chunk's tail is threaded between them
                # so the PE never waits on the serial Z/attn chain
                for pr0 in range(3):
                    if qc > 0:
                        scores_pair(pr0)
                if pending_zb is not None:
                    pending_zb()
                    pending_zb = None
                for pr0 in range(3, 7):
                    if qc > 0:
                        scores_pair(pr0)
                if pending_proj is not None:
                    pending_proj()
                    pending_proj = None
                for pr0 in range(7, 8):
                    if qc > 0:
                        scores_pair(pr0)

                u = [ps_u.tile([128, 2, 512], F32, name=f"u{h}",
                               tag=f"u{h}") for h in range(2)]
                # Z split across two engines: DVE takes odd key-tile
                # pairs, GPSIMD even ones — neither chain lags the PE
                zaccA = miscp.tile([128, 2, 512], F32, name="zaA",
                                   tag="zaA")
                zaccB = miscp.tile([128, 2, 512], F32, name="zaB",
                                   tag="zaB")

                zstash = {}

                def pv(pr, qc=qc, u=u, zaccA=zaccA, zaccB=zaccB,
                       zstash=zstash):
                    e = e_store.pop((qc, pr))
                    for ct in range(CT):
                        nc.tensor.matmul(
                            u[ct // 2][:, ct % 2, :],
                            V_f8[pr][:, :, ct * 128:(ct + 1) * 128],
                            e[:], start=(pr == 0), stop=(pr == PRS - 1),
                            perf_mode=DR)
                    # Z accumulation: fp8->f32 tensor_copy (CAST) is very
                    # slow on both GPSIMD (~3.8us) and DVE (~2.1us), so the
                    # chains start with an ADD of the first two E tiles
                    if pr % 2 == 0:
                        if pr == 0:
                            zstash[0] = e
                        elif pr == 2:
                            nc.gpsimd.tensor_add(zaccB[:], zstash.pop(0)[:],
                                                 e[:])
                        else:
                            nc.gpsimd.tensor_add(zaccB[:], zaccB[:], e[:])
                    else:
                        if pr == 1:
                            zstash[1] = e
                        elif pr == 3:
                            nc.vector.tensor_add(zaccA[:], zstash.pop(1)[:],
                                                 e[:])
                        else:
                            nc.vector.tensor_add(zaccA[:], zaccA[:], e[:])

                pre = {}
                last_qc = qc == QCHUNK - 1
                for pr in range(PRS - (2 if last_qc else 0)):
                    if qc > 0 and pr + 8 < PRS:
                        scores_pair(pr + 8)
                    pv(pr)
                if last_qc:
                    # hoist the final 2 PV steps' Z-adds + fold + column-
                    # sum + reciprocal AHEAD of the last PV matmuls: the E
                    # tiles exist well before the PV stream drains, so the
                    # softmax denominator is ready the moment U completes
                    nc.gpsimd.tensor_add(zaccB[:], zaccB[:],
                                         e_store[(qc, PRS - 2)][:])
                    nc.vector.tensor_add(zaccA[:], zaccA[:],
                                         e_store[(qc, PRS - 1)][:])

                # fold Z immediately (DVE/GPSIMD only, no PE)
                zhB = miscp.tile([128, 512], F32, name="zhB", tag="zhB")
                nc.gpsimd.tensor_add(zhB[:], zaccB[:, 0, :],
                                     zaccB[:, 1, :])
                zh = miscp.tile([128, 512], F32R, name="zh", tag="zh")
                nc.vector.tensor_add(zh[:], zaccA[:, 0, :], zaccA[:, 1, :])
                nc.vector.tensor_add(zh[:], zh[:], zhB[:])

                if last_qc:
                    zsup = ps_s.tile([128, 1024], F32, name="s", tag="s")
                    nc.tensor.matmul(zsup[:, 0:512], ones_mat[:], zh[:],
                                     start=True, stop=True)
                    rbb = miscp.tile([128, 1, 512], F32, name="rb",
                                     tag="rb")
                    nc.vector.reciprocal_approx_fast(rbb[:],
                                                     zsup[:, 0:512])
                    pre["rbb"] = rbb
                    for pr in (PRS - 2, PRS - 1):
                        e = e_store.pop((qc, pr))
                        for ct in range(CT):
                            nc.tensor.matmul(
                                u[ct // 2][:, ct % 2, :],
                                V_f8[pr][:, :, ct * 128:(ct + 1) * 128],
                                e[:], start=False, stop=(pr == PRS - 1),
                                perf_mode=DR)

                def tail_zb(qc=qc, u=u, zh=zh, pre=pre, state=None):
                    # column-sum + broadcast via the (1/ASCALE)-valued
                    # matmul; rbb = ASCALE / Z; attn = U * rbb in fp8
                    if pre:
                        rbb = pre["rbb"]
                    else:
                        zsup = ps_s.tile([128, 1024], F32, name="s",
                                         tag="s")
                        nc.tensor.matmul(zsup[:, 0:512], ones_mat[:],
                                         zh[:], start=True, stop=True)
                        rbb = miscp.tile([128, 1, 512], F32, name="rb",
                                         tag="rb")
                        nc.vector.reciprocal_approx_fast(rbb[:],
                                                         zsup[:, 0:512])
                    attn8 = [apool.tile([128, 2, 512], F8, name=f"a{p}",
                                        tag=f"a{p}") for p in range(2)]
                    state["attn8"] = attn8
                    for p in range(2):
                        nc.vector.tensor_mul(
                            attn8[p][:], u[p][:],
                            rbb[:].to_broadcast((128, 2, 512)))

                def tail_proj(qc=qc, state=None):
                    attn8 = state["attn8"]
                    # proj PSUM reuses the (drained) U banks
                    pps = [ps_u.tile([128, 1024], F32, name=f"u{h}",
                                     tag=f"u{h}") for h in range(2)]
                    last = qc == QCHUNK - 1
                    for p in range(2):
                        for h in range(2):
                            for s in range(2):
                                ot = 2 * h + s
                                nc.tensor.matmul(
                                    pps[h][:, s * 512:(s + 1) * 512],
                                    wp8[p][:, :,
                                           ot * 128:(ot + 1) * 128],
                                    attn8[p][:], start=(p == 0),
                                    stop=(p == 1), perf_mode=DR)
                    if not use_pbias:
                        # drain pps on two engines: ACT scaled-copies the
                        # s=1 halves while DVE does scale+residual stts on
                        # the s=0 halves; DVE then adds the s=1 residuals.
                        # pps is free after ~1.4us (was ~2.8us serial DVE),
                        # so the next chunk's PV matmuls unblock sooner.
                        t_os = [outp.tile([128, 1024], F32, name="out",
                                          tag="out") for h in range(2)]
                        for h in range(2):
                            nc.scalar.activation(
                                out=t_os[h][:, 512:1024],
                                in_=pps[h][:, 512:1024],
                                func=ACT.Identity,
                                scale=1.0 / (WSCALE * ASCALE))
                        sq = {(0, 0): nc.sync,
                              (1, 0): nc.scalar if last else nc.sync,
                              (0, 1): nc.gpsimd,
                              (1, 1): nc.scalar if last else nc.gpsimd}
                        for h in range(2):
                            nc.vector.scalar_tensor_tensor(
                                out=t_os[h][:, 0:512],
                                in0=pps[h][:, 0:512],
                                scalar=1.0 / (WSCALE * ASCALE),
                                in1=xa[(2 * h) * 2][
                                    :, qc * 512:(qc + 1) * 512],
                                op0=ALU.mult, op1=ALU.add)
                            for qq in range(2 if last else 1):
                                w = 256 if last else 512
                                sq[(h, 0)].dma_start(
                                    out[(2 * h) * 128:(2 * h + 1) * 128,
                                        qc * 512 + qq * w:
                                        qc * 512 + (qq + 1) * w],
                                    t_os[h][:, qq * w:(qq + 1) * w])
                        for h in range(2):
                            # split the s=1 residual adds: GPSIMD takes
                            # h0 (parallel with DVE's stts), DVE takes h1
                            # (it's free by then; GPSIMD's second add
                            # would serialize behind its first + store
                            # pushes)
                            eng = nc.gpsimd if h == 0 else nc.vector
                            eng.tensor_add(
                                t_os[h][:, 512:1024],
                                t_os[h][:, 512:1024],
                                xa[(2 * h + 1) * 2][
                                    :, qc * 512:(qc + 1) * 512])
                            for qq in range(2 if last else 1):
                                w = 256 if last else 512
                                sq[(h, 1)].dma_start(
                                    out[(2 * h + 1) * 128:
                                        (2 * h + 2) * 128,
                                        qc * 512 + qq * w:
                                        qc * 512 + (qq + 1) * w],
                                    t_os[h][:, 512 + qq * w:
                                            512 + (qq + 1) * w])
                        return
                    for h in range(2):
                        t_o = outp.tile([128, 1024], F32, name="out",
                                        tag="out")
                        if qc == QCHUNK - 1 and h == 1:
                            # final chunk h1: two per-half ACT ops run in
                            # parallel with h0's DVE work
                            for s in range(2):
                                nc.scalar.activation(
                                    out=t_o[:, s * 512:(s + 1) * 512],
                                    in_=pps[h][:, s * 512:(s + 1) * 512],
                                    func=ACT.Identity,
                                    bias=bp_t[2 * h + s],
                                    scale=1.0 / (WSCALE * ASCALE))
                        else:
                            nc.vector.scalar_tensor_tensor(
                                out=t_o[:], in0=pps[h][:],
                                scalar=1.0 / (WSCALE * ASCALE),
                                in1=bp2[h][:].to_broadcast((128, 2, 512)),
                                op0=ALU.mult, op1=ALU.add)
                        for s in range(2):
                            ot = 2 * h + s
                            nc.vector.tensor_add(
                                t_o[:, s * 512:(s + 1) * 512],
                                t_o[:, s * 512:(s + 1) * 512],
                                xa[ot * 2][:, qc * 512:(qc + 1) * 512])
                            # final chunk: split stores across both DMA
                            # queues to shorten the drain
                            dq = (nc.gpsimd if (qc == QCHUNK - 1 and h == 1)
                                  else nc.sync)
                            dq.dma_start(
                                out[ot * 128:(ot + 1) * 128,
                                    qc * 512:(qc + 1) * 512],
                                t_o[:, s * 512:(s + 1) * 512])

                def make_pending(tz=tail_zb, tp=tail_proj):
                    st = {}

                    def pz():
                        tz(state=st)

                    def pp_():
                        tp(state=st)
                    return pz, pp_

                pending_zb, pending_proj = make_pending()
            pending_zb()
            pending_proj()


# ---------------- host-side sharding / gather ----------------

_CACHED_NC = {}


def _get_nc(use_pbias=False):
    if use_pbias not in _CACHED_NC:
        _CACHED_NC[use_pbias] = build_module(reps=1, use_pbias=use_pbias)
    return _CACHED_NC[use_pbias]


def _make_in_maps(x, gn_w, gn_b, qkv_w, qkv_b, proj_w, proj_b):
    b, c, h, w = x.shape
    n = h * w
    assert (b, c, n) == (4, C, N)
    xr = np.ascontiguousarray(x.reshape(b, c, n)).astype(np.float32)
    xr16 = xr.astype(NPBF)

    # fp8 weights, prescaled x16.  No 1/sqrt(c) folding: that lives in the
    # exp activation's scale.
    w8_h = np.ascontiguousarray(
        np.concatenate([qkv_w[0:c].T, qkv_w[c:2 * c].T, qkv_w[2 * c:3 * c].T],
                       axis=1) * WSCALE).astype(NP8)
    wp_h = np.ascontiguousarray(proj_w.T * WSCALE).astype(NP8)

    bq_h = np.asarray(qkv_b[0:c], np.float32).reshape(CT, 128)
    bk_h = np.asarray(qkv_b[c:2 * c], np.float32).reshape(CT, 128)
    # v-bias folded through the projection:  proj(attn + bv) =
    # proj(attn) + proj_w @ bv, so it lands in the proj bias.
    bp_eff = (np.asarray(proj_b, np.float64)
              + np.asarray(proj_w, np.float64) @ np.asarray(
                  qkv_b[2 * c:3 * c], np.float64)).astype(np.float32)
    bp_h = bp_eff.reshape(CT, 128)
    gnw_h = np.asarray(gn_w, np.float32).reshape(CT, 128)
    gnb_h = np.asarray(gn_b, np.float32).reshape(CT, 128)
    pidx = np.arange(128)
    indr_h = (pidx[:, None] // GSIZE == np.arange(8)[None, :]).astype(
        np.float32) / GSIZE
    gmask_h = np.ascontiguousarray(
        (pidx[:, None] // GSIZE == pidx[None, :] // GSIZE).astype(
            np.float32) / GSIZE)
    cvec_h = np.zeros((128, 28), np.float32)
    for ct in range(CT):
        cvec_h[:, ct] = bq_h[ct]
        cvec_h[:, 4 + ct] = bk_h[ct]
        cvec_h[:, 8 + ct] = bp_h[ct]
        cvec_h[:, 12 + ct] = gnw_h[ct]
        cvec_h[:, 16 + ct] = gnb_h[ct]
    cvec_h[:, 20:28] = indr_h

    shared = dict(w8=w8_h, wpd=wp_h, cvec=cvec_h, gmask=gmask_h)
    in_maps = []
    for core in range(N_CORES):
        bi, half = core // 2, core % 2
        xb = xr16[bi]
        if half:
            xb = np.ascontiguousarray(
                np.concatenate([xb[:, NQ:], xb[:, :NQ]], axis=1))
        in_maps.append({"xin": xb, **shared})
    return in_maps


def kernel(x, gn_w, gn_b, qkv_w, qkv_b, proj_w, proj_b):
    bp_eff = (np.asarray(proj_b, np.float64)
              + np.asarray(proj_w, np.float64) @ np.asarray(
                  qkv_b[2 * C:3 * C], np.float64))
    nc = _get_nc(use_pbias=bool(np.any(bp_eff != 0.0)))
    in_maps = _make_in_maps(x, gn_w, gn_b, qkv_w, qkv_b, proj_w, proj_b)
    res = run_bass_kernel_spmd(nc, in_maps, list(range(N_CORES)))
    b, c, h, w = x.shape
    out_full = np.empty((b, C, N), dtype=np.float32)
    for core in range(N_CORES):
        bi, half = core // 2, core % 2
        out_full[bi, :, half * NQ:(half + 1) * NQ] = res.results[core]["out"]
    return out_full.reshape(b, c, h, w)

